# revision 4
# baseline (speedup 1.0000x reference)
"""LoRA layer (rank-16) Trainium2 kernel.

Computes y = dropout(x @ (B@A).T * scaling) for x [4, 4096, 4096],
lora_A [16, 4096], lora_B [4096, 16], using the low-rank factorization
(x @ A.T) @ B.T instead of materializing the 4096x4096 weight.

Distribution: data-parallel over the 16384 tokens across 8 NeuronCores
(2048 tokens/core); A/B are tiny and replicated. The dropout mask is the
deterministic jax.random.bernoulli(key(42)) tensor from the reference --
precomputed on host CPU, shipped as float8 {0.0, 1.0}, and applied on-device
as the PSUM->SBUF eviction multiply.

Device-side layout: host pre-transposes x so each core receives
xT [4096 in_features, 2048 tokens] -- the contraction dim lands on SBUF
partitions for matmul #1 with zero on-chip transposes, and matmul #1's
PSUM output tT [16, tokens] is directly the stationary operand layout
for matmul #2, whose output is natural-layout y [tokens, out_features].
"""

import os

import numpy as np
import ml_dtypes

import concourse.bass as bass
import concourse.mybir as mybir
import concourse.tile as tile
from concourse.bass_utils import run_bass_kernel_spmd

RANK = 16
SCALING = 32.0 / 16.0
KEEP = 0.9
IN_F = 4096
OUT_F = 4096
BATCH = 4
SEQ = 4096
TOKENS = BATCH * SEQ          # 16384
N_CORES = 8
TPC = TOKENS // N_CORES       # 2048 tokens per core
TC = 256                      # token chunk per pipeline step
N_CHUNKS = TPC // TC          # 8
KT = IN_F // 128              # 32 k-tiles for matmul #1
NF = 512                      # out_features tile for matmul #2 (1 PSUM bank fp32)
MT = TC // 128                # token m-tiles per chunk

# matmul input dtype: "f32", "f32r" (same bits, fast PE path), or "bf16"
MM_DTYPE = os.environ.get("KERNEL_MM_DTYPE", "f32r")

_FP8 = ml_dtypes.float8_e4m3


def _legalize_waits(nc: bass.Bass, max_waits: int = 1) -> None:
    """This container's walrus accepts at most one sync wait per instruction
    (codegen 'Too many sync wait commands', e.g. on Tile's tail Drain or on
    fused fp32 matmuls). Hoist surplus waits onto same-engine NoOps inserted
    immediately before the offending instruction -- semantics preserved since
    waits execute in engine program order either way."""
    n = 0
    for f in nc.m.functions:
        for b in f.blocks:
            changed = False
            new = []
            for inst in b.instructions:
                si = inst.sync_info
                waits = list(si.on_wait) if si is not None and si.on_wait else []
                if len(waits) > max_waits:
                    changed = True
                    for j, w in enumerate(waits):
                        n += 1
                        new.append(
                            mybir.InstNoOp(
                                name=f"{inst.name}-wait{j}",
                                engine=inst.engine,
                                ins=[],
                                outs=[],
                                sync_info=mybir.SyncInfo(on_wait=[w], on_update=[]),
                            )
                        )
                    inst.sync_info = mybir.SyncInfo(
                        on_wait=[], on_update=list(si.on_update or [])
                    )
                new.append(inst)
            if changed:
                b.instructions = new


def _build_bass(mm_dtype: str) -> bass.Bass:
    if mm_dtype == "bf16":
        dt_x = mybir.dt.bfloat16
    elif mm_dtype == "f32r":
        # FP32r: fp32 bits on the fast (1 cycle/row) PE path. The verifier
        # requires every matmul operand to be produced AS float32r, so the
        # whole x/A/B/t dataflow is declared float32r (numpy side: float32).
        dt_x = mybir.dt.float32r
    else:
        dt_x = mybir.dt.float32

    def mm_ap(ap):
        return ap

    nc = bass.Bass()
    xT = nc.dram_tensor("xT", [IN_F, TPC], dt_x, kind="ExternalInput")
    at = nc.dram_tensor("AT", [IN_F, RANK], dt_x, kind="ExternalInput")
    bt = nc.dram_tensor("BT", [RANK, OUT_F], dt_x, kind="ExternalInput")
    mk = nc.dram_tensor("mask", [TPC, OUT_F], mybir.dt.float8e4, kind="ExternalInput")
    y = nc.dram_tensor("y", [TPC, OUT_F], mybir.dt.float32, kind="ExternalOutput")

    with tile.TileContext(nc) as tc:
        with (
            tc.tile_pool(name="const", bufs=1) as cpool,
            tc.tile_pool(name="xin", bufs=2) as xpool,
            tc.tile_pool(name="mkin", bufs=2) as mpool,
            tc.tile_pool(name="yout", bufs=2) as ypool,
            tc.tile_pool(name="tsb", bufs=3) as tpool,
            tc.tile_pool(name="pst", bufs=2, space="PSUM") as pst,
            tc.tile_pool(name="psy", bufs=6, space="PSUM") as psy,
        ):
            at_t = cpool.tile([128, KT, RANK], dt_x)
            nc.sync.dma_start(at_t[:], at.rearrange("(kt p) m -> p kt m", p=128))
            bt_t = cpool.tile([RANK, OUT_F], dt_x)
            nc.sync.dma_start(bt_t[:], bt[:])

            for c in range(N_CHUNKS):
                tok = c * TC
                xt_t = xpool.tile([128, KT, TC], dt_x)
                nc.sync.dma_start(
                    xt_t[:],
                    xT[:, tok : tok + TC].rearrange("(kt p) t -> p kt t", p=128),
                )
                mk_t = mpool.tile([128, MT, OUT_F], mybir.dt.float8e4)
                nc.sync.dma_start(
                    mk_t[:],
                    mk[tok : tok + TC, :].rearrange("(mt p) f -> p mt f", p=128),
                )

                ps_t = pst.tile([RANK, TC], mybir.dt.float32)
                for k in range(KT):
                    nc.tensor.matmul(
                        ps_t[:],
                        mm_ap(at_t[:, k, :]),
                        mm_ap(xt_t[:, k, :]),
                        start=(k == 0),
                        stop=(k == KT - 1),
                    )
                t_sb = tpool.tile([RANK, TC], dt_x)
                nc.scalar.copy(t_sb[:], ps_t[:])

                y_ch = ypool.tile([128, MT, OUT_F], mybir.dt.float32)
                for m in range(MT):
                    for n in range(OUT_F // NF):
                        ps_y = psy.tile([128, NF], mybir.dt.float32)
                        nc.tensor.matmul(
                            ps_y[:],
                            mm_ap(t_sb[:, m * 128 : (m + 1) * 128]),
                            mm_ap(bt_t[:, n * NF : (n + 1) * NF]),
                            start=True,
                            stop=True,
                        )
                        nc.vector.tensor_tensor(
                            y_ch[:, m, n * NF : (n + 1) * NF],
                            ps_y[:],
                            mk_t[:, m, n * NF : (n + 1) * NF],
                            op=mybir.AluOpType.mult,
                        )
                nc.sync.dma_start(
                    y[tok : tok + TC, :].rearrange("(mt p) f -> p mt f", p=128),
                    y_ch[:],
                )
    _legalize_waits(nc)
    return nc


def _dropout_mask() -> np.ndarray:
    """The reference's deterministic dropout mask as float8 {0.0, 1.0}."""
    import jax

    cpu = jax.local_devices(backend="cpu")[0]
    with jax.default_device(cpu):
        keep = jax.random.bernoulli(jax.random.key(42), KEEP, (TOKENS, OUT_F))
        keep = np.asarray(keep)
    return keep.astype(_FP8)


_BUILT = {}


def kernel(x: np.ndarray, lora_A: np.ndarray, lora_B: np.ndarray) -> np.ndarray:
    mm_dtype = MM_DTYPE
    np_x = ml_dtypes.bfloat16 if mm_dtype == "bf16" else np.float32

    x_flat = np.asarray(x, dtype=np.float32).reshape(TOKENS, IN_F)
    # scaling and the inverted-dropout 1/keep both fold into B
    at_host = np.ascontiguousarray(np.asarray(lora_A, np.float32).T).astype(np_x)
    bt_host = np.ascontiguousarray(
        np.asarray(lora_B, np.float32).T * (SCALING / KEEP)
    ).astype(np_x)
    mask = _dropout_mask()

    in_maps = []
    for i in range(N_CORES):
        shard = x_flat[i * TPC : (i + 1) * TPC]
        in_maps.append(
            {
                "xT": np.ascontiguousarray(shard.T).astype(np_x),
                "AT": at_host,
                "BT": bt_host,
                "mask": mask[i * TPC : (i + 1) * TPC],
            }
        )

    if mm_dtype not in _BUILT:
        _BUILT[mm_dtype] = _build_bass(mm_dtype)
    nc = _BUILT[mm_dtype]

    trace = bool(int(os.environ.get("KERNEL_TRACE", "0")))
    res = run_bass_kernel_spmd(
        nc, in_maps, core_ids=list(range(N_CORES)), trace=trace
    )
    kernel.last_results = res

    out = np.concatenate([r["y"] for r in res.results], axis=0)
    return out.reshape(BATCH, SEQ, OUT_F)


# revision 7
# speedup vs baseline: 1.2177x; 1.2177x over previous
"""LoRA layer (rank-16) Trainium2 kernel.

Computes y = dropout(x @ (B@A).T * scaling) for x [4, 4096, 4096],
lora_A [16, 4096], lora_B [4096, 16], using the low-rank factorization
(x @ A.T) @ B.T instead of materializing the 4096x4096 weight.

Distribution: data-parallel over the 16384 tokens across 8 NeuronCores
(2048 tokens/core); A/B are tiny and replicated. The dropout mask is the
deterministic jax.random.bernoulli(key(42)) tensor from the reference --
precomputed on host CPU, shipped as float8 {0.0, 1.0}, and applied on-device
as the PSUM->SBUF eviction multiply.

Device-side layout: host pre-transposes x so each core receives
xT [4096 in_features, 2048 tokens] -- the contraction dim lands on SBUF
partitions for matmul #1 with zero on-chip transposes, and matmul #1's
PSUM output tT [16, tokens] is directly the stationary operand layout
for matmul #2, whose output is natural-layout y [tokens, out_features].
"""

import os

import numpy as np
import ml_dtypes

import concourse.bass as bass
import concourse.mybir as mybir
import concourse.tile as tile
from concourse.bass_utils import run_bass_kernel_spmd

RANK = 16
SCALING = 32.0 / 16.0
KEEP = 0.9
IN_F = 4096
OUT_F = 4096
BATCH = 4
SEQ = 4096
TOKENS = BATCH * SEQ          # 16384
N_CORES = 8
TPC = TOKENS // N_CORES       # 2048 tokens per core
TC = 256                      # token chunk per pipeline step
N_CHUNKS = TPC // TC          # 8
KT = IN_F // 128              # 32 k-tiles for matmul #1
NF = 512                      # out_features tile for matmul #2 (1 PSUM bank fp32)
MT = TC // 128                # token m-tiles per chunk

# matmul input dtype: "f32", "f32r", "bf16", or "fp16"
# fp16 measured best: 1 cycle/row PE speed (fp32/f32r run 4x slower), half the
# x DMA bytes of fp32, and norm rel err ~4e-4 (bf16: 3e-3; fp32: 4e-7).
MM_DTYPE = os.environ.get("KERNEL_MM_DTYPE", "fp16")

_FP8 = ml_dtypes.float8_e4m3


def _legalize_waits(nc: bass.Bass, max_waits: int = 1) -> None:
    """This container's walrus accepts at most one sync wait per instruction
    (codegen 'Too many sync wait commands', e.g. on Tile's tail Drain or on
    fused fp32 matmuls). Hoist surplus waits onto same-engine NoOps inserted
    immediately before the offending instruction -- semantics preserved since
    waits execute in engine program order either way."""
    n = 0
    for f in nc.m.functions:
        for b in f.blocks:
            changed = False
            new = []
            for inst in b.instructions:
                si = inst.sync_info
                waits = list(si.on_wait) if si is not None and si.on_wait else []
                if len(waits) > max_waits:
                    changed = True
                    for j, w in enumerate(waits):
                        n += 1
                        new.append(
                            mybir.InstNoOp(
                                name=f"{inst.name}-wait{j}",
                                engine=inst.engine,
                                ins=[],
                                outs=[],
                                sync_info=mybir.SyncInfo(on_wait=[w], on_update=[]),
                            )
                        )
                    inst.sync_info = mybir.SyncInfo(
                        on_wait=[], on_update=list(si.on_update or [])
                    )
                new.append(inst)
            if changed:
                b.instructions = new


def _build_bass(mm_dtype: str) -> bass.Bass:
    if mm_dtype == "bf16":
        dt_x = mybir.dt.bfloat16
    elif mm_dtype == "fp16":
        dt_x = mybir.dt.float16
    elif mm_dtype == "f32r":
        # FP32r: fp32 bits on the fast (1 cycle/row) PE path. The verifier
        # requires every matmul operand to be produced AS float32r, so the
        # whole x/A/B/t dataflow is declared float32r (numpy side: float32).
        dt_x = mybir.dt.float32r
    else:
        dt_x = mybir.dt.float32

    def mm_ap(ap):
        return ap

    nc = bass.Bass()
    xT = nc.dram_tensor("xT", [IN_F, TPC], dt_x, kind="ExternalInput")
    at = nc.dram_tensor("AT", [IN_F, RANK], dt_x, kind="ExternalInput")
    bt = nc.dram_tensor("BT", [RANK, OUT_F], dt_x, kind="ExternalInput")
    mk = nc.dram_tensor("mask", [TPC, OUT_F], mybir.dt.float8e4, kind="ExternalInput")
    y = nc.dram_tensor("y", [TPC, OUT_F], mybir.dt.float32, kind="ExternalOutput")

    with tile.TileContext(nc) as tc:
        with (
            tc.tile_pool(name="const", bufs=1) as cpool,
            tc.tile_pool(name="xin", bufs=2) as xpool,
            tc.tile_pool(name="mkin", bufs=2) as mpool,
            tc.tile_pool(name="yout", bufs=2) as ypool,
            tc.tile_pool(name="tsb", bufs=3) as tpool,
            tc.tile_pool(name="pst", bufs=2, space="PSUM") as pst,
            tc.tile_pool(name="psy", bufs=6, space="PSUM") as psy,
        ):
            at_t = cpool.tile([128, KT, RANK], dt_x)
            nc.sync.dma_start(at_t[:], at.rearrange("(kt p) m -> p kt m", p=128))
            bt_t = cpool.tile([RANK, OUT_F], dt_x)
            nc.sync.dma_start(bt_t[:], bt[:])

            for c in range(N_CHUNKS):
                tok = c * TC
                xt_t = xpool.tile([128, KT, TC], dt_x)
                nc.sync.dma_start(
                    xt_t[:],
                    xT[:, tok : tok + TC].rearrange("(kt p) t -> p kt t", p=128),
                )
                mk_t = mpool.tile([128, MT, OUT_F], mybir.dt.float8e4)
                nc.sync.dma_start(
                    mk_t[:],
                    mk[tok : tok + TC, :].rearrange("(mt p) f -> p mt f", p=128),
                )

                ps_t = pst.tile([RANK, TC], mybir.dt.float32)
                for k in range(KT):
                    nc.tensor.matmul(
                        ps_t[:],
                        mm_ap(at_t[:, k, :]),
                        mm_ap(xt_t[:, k, :]),
                        start=(k == 0),
                        stop=(k == KT - 1),
                    )
                t_sb = tpool.tile([RANK, TC], dt_x)
                nc.scalar.copy(t_sb[:], ps_t[:])

                y_ch = ypool.tile([128, MT, OUT_F], mybir.dt.float32)
                for m in range(MT):
                    for n in range(OUT_F // NF):
                        ps_y = psy.tile([128, NF], mybir.dt.float32)
                        nc.tensor.matmul(
                            ps_y[:],
                            mm_ap(t_sb[:, m * 128 : (m + 1) * 128]),
                            mm_ap(bt_t[:, n * NF : (n + 1) * NF]),
                            start=True,
                            stop=True,
                        )
                        nc.vector.tensor_tensor(
                            y_ch[:, m, n * NF : (n + 1) * NF],
                            ps_y[:],
                            mk_t[:, m, n * NF : (n + 1) * NF],
                            op=mybir.AluOpType.mult,
                        )
                nc.sync.dma_start(
                    y[tok : tok + TC, :].rearrange("(mt p) f -> p mt f", p=128),
                    y_ch[:],
                )
    _legalize_waits(nc)
    return nc


def _dropout_mask() -> np.ndarray:
    """The reference's deterministic dropout mask as float8 {0.0, 1.0}."""
    import jax

    cpu = jax.local_devices(backend="cpu")[0]
    with jax.default_device(cpu):
        keep = jax.random.bernoulli(jax.random.key(42), KEEP, (TOKENS, OUT_F))
        keep = np.asarray(keep)
    return keep.astype(_FP8)


_BUILT = {}


def kernel(x: np.ndarray, lora_A: np.ndarray, lora_B: np.ndarray) -> np.ndarray:
    mm_dtype = MM_DTYPE
    np_x = {"bf16": ml_dtypes.bfloat16, "fp16": np.float16}.get(mm_dtype, np.float32)

    x_flat = np.asarray(x, dtype=np.float32).reshape(TOKENS, IN_F)
    # scaling and the inverted-dropout 1/keep both fold into B
    at_host = np.ascontiguousarray(np.asarray(lora_A, np.float32).T).astype(np_x)
    bt_host = np.ascontiguousarray(
        np.asarray(lora_B, np.float32).T * (SCALING / KEEP)
    ).astype(np_x)
    mask = _dropout_mask()

    in_maps = []
    for i in range(N_CORES):
        shard = x_flat[i * TPC : (i + 1) * TPC]
        in_maps.append(
            {
                "xT": np.ascontiguousarray(shard.T).astype(np_x),
                "AT": at_host,
                "BT": bt_host,
                "mask": mask[i * TPC : (i + 1) * TPC],
            }
        )

    if mm_dtype not in _BUILT:
        _BUILT[mm_dtype] = _build_bass(mm_dtype)
    nc = _BUILT[mm_dtype]

    trace = bool(int(os.environ.get("KERNEL_TRACE", "0")))
    res = run_bass_kernel_spmd(
        nc, in_maps, core_ids=list(range(N_CORES)), trace=trace
    )
    kernel.last_results = res

    out = np.concatenate([r["y"] for r in res.results], axis=0)
    return out.reshape(BATCH, SEQ, OUT_F)
